# revision 28
# baseline (speedup 1.0000x reference)
"""KernelNorm2d Trainium2 Bass kernel (fp16 I/O, window-major layout).

Problem: x [16, 64, 256, 256] f32. 2x2 windows (stride 2) over (H, W); per-window
statistics over (C, 2, 2) = 256 elements; out = (x - mean) / sqrt(var + eps).
Data-parallel over batch: 8 cores x 2 samples each.

Host relayouts x to window-major [B, nH, nW, (c a b)] fp16, so each window's 256
elements are contiguous in SBUF (partition = window row i). DMA runs are 32 KiB
per partition; loads and stores are split across the two HW DGE rings
(sync carries sample-0 loads then late stores, scalar carries sample-1 loads
and early stores).

Stats: mostly ONE DVE pass via bn_stats (one instr per window, 6-tuple for the
two equal 128-elem groups); a slice of windows per unit is offloaded to ACT
(Identity+accum_out = sum, Square+accum_out = sumsq) to relieve the DVE, which
is the bottleneck engine. Both paths produce 256*var in a shared tile, one ACT
sqrt(x/256 + eps) + DVE reciprocal per unit yields istd. Normalize is
per-window-column scale+bias split across ACT/GPSIMD steady-state, with the
pipeline tail (last units) on DVE+GPSIMD so the final sqrts never queue behind
ACT normalize columns.

Measured instruction facts (HW traces): bn_stats 327 ns/window (no DVE fast
mode exists for it); per-col normalize: DVE 305 ns, ACT 495 ns, GPSIMD 537 ns;
ACT per-window accum stats ~1.0 us; HW DGE ring ~225 GB/s, 2 rings.
"""

import os
import sys

for _p in ("/opt/trn_rl_repo", "/root/.axon_site/_ro/trn_rl_repo"):
    if os.path.isdir(_p) and _p not in sys.path:
        sys.path.append(_p)

import numpy as np

import concourse.bass as bass
import concourse.tile as tile
from concourse import bacc, mybir
from concourse.bass_utils import run_bass_kernel_spmd

# Problem constants (hardcoded per spec nn_KernelNorm2d_72164040507639)
B, C, H, W = 16, 64, 256, 256
N_CORES = 8
B_LOC = B // N_CORES          # samples per core
NH = H // 2                   # 128 window rows = partition dim
NJ = W // 2                   # 128 window cols
WIN = C * 4                   # 256 elements per window
EPS = 1e-5


def _make_pattern(nv, ns, n):
    w = {"v": nv, "s": ns, "g": n - nv - ns}
    acc = {"v": 0.0, "s": 0.0, "g": 0.0}
    pat = []
    for k in range(n):
        best = max(w, key=lambda e: w[e] / n * (k + 1) - acc[e])
        acc[best] += 1
        pat.append(best)
    return "".join(pat)


# ACT share is front-loaded: heavy early (its istd arrives while DVE still
# streams bn), light late (so the final sqrts never sit behind queued columns).
P8 = _make_pattern(0, 5, 8)
P16 = _make_pattern(0, 9, 16)
P32 = _make_pattern(0, 20, 32)
P64A = _make_pattern(0, 40, 64)   # early half
P64B = _make_pattern(0, 30, 64)   # later half
P32L = _make_pattern(0, 8, 32)    # late quarter: mostly GPSIMD
P16M = _make_pattern(8, 0, 16)    # second-to-last: DVE+GPSIMD
P16T = _make_pattern(8, 4, 16)    # tail: all three, small shares


def build_kernel(debug: bool = False) -> bass.Bass:
    nc = bacc.Bacc("TRN2", debug=debug)
    f16 = mybir.dt.float16
    f32 = mybir.dt.float32
    x = nc.dram_tensor("x", [B_LOC, NH, NJ, WIN], f16, kind="ExternalInput")
    y = nc.dram_tensor("y", [B_LOC, NH, NJ, WIN], f16, kind="ExternalOutput")

    with tile.TileContext(nc) as tc:
        with (
            tc.tile_pool(name="data", bufs=2) as data_pool,
            tc.tile_pool(name="stats", bufs=2) as stats_pool,
            tc.tile_pool(name="singles", bufs=1) as singles,
        ):
            eps_tile = singles.tile([NH, 1], f32)
            nc.vector.memset(eps_tile, EPS)
            dump_g = singles.tile([NH, WIN], f16)

            state = {}

            def stats(b, js, jn, xt, ko=0):
                """per-window stats for cols [js, js+jn): bn_stats on DVE for
                the first jn-ko cols, GPSIMD accum (sum+sumsq) for the last ko
                (only in units where GPSIMD has idle ahead of the combine)."""
                tg = f"{js}_{jn}"
                kb = jn - ko
                S = stats_pool.tile([NH, kb, 2, 3], f32, tag=f"S{tg}")
                var = stats_pool.tile([NH, jn], f32, tag=f"var{tg}")
                istd = stats_pool.tile([NH, jn], f32, tag=f"istd{tg}")
                tsh = stats_pool.tile([NH, jn], f32, tag=f"tsh{tg}")
                if ko:
                    osum = stats_pool.tile([NH, ko], f32, tag=f"osum{tg}")
                    osq = stats_pool.tile([NH, ko], f32, tag=f"osq{tg}")
                    u2 = stats_pool.tile([NH, ko], f32, tag=f"u2{tg}")
                    for i in range(ko):
                        win = xt[:, js + kb + i, :]
                        nc.gpsimd.tensor_scalar(
                            out=win, in0=win, scalar1=1.0, scalar2=0.0,
                            op0=mybir.AluOpType.mult,
                            op1=mybir.AluOpType.add,
                            accum_out=osum[:, i : i + 1],
                        )
                        nc.gpsimd.scalar_tensor_tensor(
                            out=dump_g, in0=win, scalar=1.0, in1=win,
                            op0=mybir.AluOpType.mult, op1=mybir.AluOpType.mult,
                            accum_out=osq[:, i : i + 1],
                        )
                for t in range(kb):
                    nc.vector.bn_stats(out=S[:, t], in_=xt[:, js + t, :])
                # two equal 128-elem groups: mu = (m0+m1)/2,
                # 256*var = (cv0+cv1) + 64*(m0-m1)^2
                m0, m1 = S[:, :, 0, 1], S[:, :, 1, 1]
                sm = stats_pool.tile([NH, kb], f32, tag=f"sm{tg}")
                sv = stats_pool.tile([NH, kb], f32, tag=f"sv{tg}")
                d = stats_pool.tile([NH, kb], f32, tag=f"d{tg}")
                nc.vector.tensor_add(out=sm, in0=m0, in1=m1)
                nc.vector.tensor_add(out=sv, in0=S[:, :, 0, 2], in1=S[:, :, 1, 2])
                nc.vector.tensor_tensor(
                    out=d, in0=m0, in1=m1, op=mybir.AluOpType.subtract
                )
                nc.vector.tensor_mul(out=d, in0=d, in1=d)
                nc.vector.scalar_tensor_tensor(
                    out=var[:, :kb], in0=d, scalar=64.0, in1=sv,
                    op0=mybir.AluOpType.mult, op1=mybir.AluOpType.add,
                )
                if ko:
                    nc.vector.tensor_mul(out=u2, in0=osum, in1=osum)
                    nc.vector.scalar_tensor_tensor(
                        out=var[:, kb:], in0=u2, scalar=-1.0 / WIN, in1=osq,
                        op0=mybir.AluOpType.mult, op1=mybir.AluOpType.add,
                    )
                # istd = 1/sqrt(var + eps); tsh = -mu * istd
                nc.scalar.activation(
                    out=var, in_=var, func=mybir.ActivationFunctionType.Sqrt,
                    bias=eps_tile, scale=1.0 / WIN,
                )
                nc.vector.reciprocal(out=istd, in_=var)
                nc.vector.scalar_tensor_tensor(
                    out=tsh[:, :kb], in0=sm, scalar=-0.5, in1=istd[:, :kb],
                    op0=mybir.AluOpType.mult, op1=mybir.AluOpType.mult,
                )
                if ko:
                    nc.vector.scalar_tensor_tensor(
                        out=tsh[:, kb:], in0=osum, scalar=-1.0 / WIN,
                        in1=istd[:, kb:],
                        op0=mybir.AluOpType.mult, op1=mybir.AluOpType.mult,
                    )
                state[(b, js)] = (xt, istd, tsh)

            def normalize(b, js, jn, pattern, store_q):
                """normalize cols [js, js+jn) in place, then store them."""
                xt, istd, tsh = state.pop((b, js))
                for jo in range(jn):
                    win = xt[:, js + jo, :]
                    eng = pattern[jo]
                    if eng == "s":
                        nc.scalar.activation(
                            out=win,
                            in_=win,
                            func=mybir.ActivationFunctionType.Identity,
                            bias=tsh[:, jo : jo + 1],
                            scale=istd[:, jo : jo + 1],
                        )
                    else:
                        e = nc.vector if eng == "v" else nc.gpsimd
                        e.tensor_scalar(
                            out=win,
                            in0=win,
                            scalar1=istd[:, jo : jo + 1],
                            scalar2=tsh[:, jo : jo + 1],
                            op0=mybir.AluOpType.mult,
                            op1=mybir.AluOpType.add,
                        )
                store_q.dma_start(
                    out=y[b, :, js : js + jn], in_=xt[:, js : js + jn]
                )

            # software-pipelined units; small at the head (early DVE start)
            # and tail (short drain). All loads on the sync ring (in pipeline
            # order); stores on scalar, late stores on the by-then-idle sync.
            xt0 = data_pool.tile([NH, NJ, WIN], f16, tag="xt")
            xt1 = data_pool.tile([NH, NJ, WIN], f16, tag="xt")
            for xt, b, js, jn in (
                (xt0, 0, 0, 8),
                (xt0, 0, 8, 8),
                (xt0, 0, 16, 16),
                (xt0, 0, 32, 32),
                (xt0, 0, 64, 64),
                (xt1, 1, 0, 64),
                (xt1, 1, 64, 64),
            ):
                nc.sync.dma_start(
                    out=xt[:, js : js + jn], in_=x[b, :, js : js + jn]
                )
            units = [
                (0, 0, 8, 0, P8, nc.scalar),
                (0, 8, 8, 0, P8, nc.scalar),
                (0, 16, 16, 0, P16, nc.scalar),
                (0, 32, 32, 0, P32, nc.scalar),
                (0, 64, 64, 0, P64A, nc.scalar),
                (1, 0, 64, 0, P64B, nc.sync),
                (1, 64, 32, 0, P32L, nc.scalar),
                (1, 96, 16, 0, P16M, nc.scalar),
                (1, 112, 16, 0, P16T, nc.sync),
            ]
            for b, js, jn, ko, pat, q in units:
                stats(b, js, jn, xt0 if b == 0 else xt1, ko)
                normalize(b, js, jn, pat, q)
    nc.compile()
    return nc


_NC_CACHE = None
LAST_RESULTS = None


def _get_nc():
    global _NC_CACHE
    if _NC_CACHE is None:
        _NC_CACHE = build_kernel()
    return _NC_CACHE


def kernel(x: np.ndarray) -> np.ndarray:
    global LAST_RESULTS
    assert x.shape == (B, C, H, W), x.shape
    # window-major host relayout: [B, C, H, W] -> [B, nH, nW, (c a b)] fp16
    xh = np.ascontiguousarray(
        x.astype(np.float16)
        .reshape(B, C, NH, 2, NJ, 2)
        .transpose(0, 2, 4, 1, 3, 5)
        .reshape(B, NH, NJ, WIN)
    )
    nc = _get_nc()
    in_maps = [{"x": xh[k * B_LOC : (k + 1) * B_LOC]} for k in range(N_CORES)]
    kw = {}
    if os.environ.get("KERNEL_TRACE") == "1":
        kw["trace"] = True
        if os.environ.get("KERNEL_TRACE_DIR"):
            import tempfile

            base = os.environ["KERNEL_TRACE_DIR"]
            os.makedirs(base, exist_ok=True)
            kw["tmpdir"] = tempfile.mkdtemp(dir=base)
    res = run_bass_kernel_spmd(nc, in_maps, core_ids=list(range(N_CORES)), **kw)
    LAST_RESULTS = res
    out = np.concatenate([r["y"] for r in res.results], axis=0)
    return (
        out.reshape(B, NH, NJ, C, 2, 2)
        .transpose(0, 3, 1, 4, 2, 5)
        .reshape(B, C, H, W)
        .astype(np.float32)
    )


# revision 30
# speedup vs baseline: 1.2374x; 1.2374x over previous
"""KernelNorm2d Trainium2 Bass kernel (fp16 I/O, window-major layout).

Problem: x [16, 64, 256, 256] f32. 2x2 windows (stride 2) over (H, W); per-window
statistics over (C, 2, 2) = 256 elements; out = (x - mean) / sqrt(var + eps).
Data-parallel over batch: 8 cores x 2 samples each.

Host relayouts x to window-major [B, nH, nW, (c a b)] fp16, so each window's 256
elements are contiguous in SBUF (partition = window row i). DMA runs are 32 KiB
per partition; loads and stores are split across the two HW DGE rings
(sync carries sample-0 loads then late stores, scalar carries sample-1 loads
and early stores).

Stats: mostly ONE DVE pass via bn_stats (one instr per window, 6-tuple for the
two equal 128-elem groups); a slice of windows per unit is offloaded to ACT
(Identity+accum_out = sum, Square+accum_out = sumsq) to relieve the DVE, which
is the bottleneck engine. Both paths produce 256*var in a shared tile, one ACT
sqrt(x/256 + eps) + DVE reciprocal per unit yields istd. Normalize is
per-window-column scale+bias split across ACT/GPSIMD steady-state, with the
pipeline tail (last units) on DVE+GPSIMD so the final sqrts never queue behind
ACT normalize columns.

Measured instruction facts (HW traces): bn_stats 327 ns/window (no DVE fast
mode exists for it); per-col normalize: DVE 305 ns, ACT 495 ns, GPSIMD 537 ns;
ACT per-window accum stats ~1.0 us; HW DGE ring ~225 GB/s, 2 rings.
"""

import os
import sys

for _p in ("/opt/trn_rl_repo", "/root/.axon_site/_ro/trn_rl_repo"):
    if os.path.isdir(_p) and _p not in sys.path:
        sys.path.append(_p)

import numpy as np

import concourse.bass as bass
import concourse.tile as tile
from concourse import bacc, mybir
from concourse.bass_utils import run_bass_kernel_spmd

# Problem constants (hardcoded per spec nn_KernelNorm2d_72164040507639)
B, C, H, W = 16, 64, 256, 256
N_CORES = 8
B_LOC = B // N_CORES          # samples per core
NH = H // 2                   # 128 window rows = partition dim
NJ = W // 2                   # 128 window cols
WIN = C * 4                   # 256 elements per window
EPS = 1e-5


def _make_pattern(nv, ns, n):
    w = {"v": nv, "s": ns, "g": n - nv - ns}
    acc = {"v": 0.0, "s": 0.0, "g": 0.0}
    pat = []
    for k in range(n):
        best = max(w, key=lambda e: w[e] / n * (k + 1) - acc[e])
        acc[best] += 1
        pat.append(best)
    return "".join(pat)


# NOTE: concentrating ACT/GPSIMD work to overlap the DVE bn stream harder
# hits a shared SBUF bandwidth ceiling (all per-op costs inflate ~20%);
# the staggered v7b-style split below measures fastest.
P8 = _make_pattern(0, 5, 8)
P16 = _make_pattern(0, 9, 16)
P32 = _make_pattern(0, 17, 32)
P64 = _make_pattern(0, 33, 64)
P16L = _make_pattern(0, 4, 16)    # second-to-last: light on ACT
P16T = _make_pattern(6, 5, 16)    # tail: all three engines


def build_kernel(debug: bool = False) -> bass.Bass:
    nc = bacc.Bacc("TRN2", debug=debug)
    f16 = mybir.dt.float16
    f32 = mybir.dt.float32
    x = nc.dram_tensor("x", [B_LOC, NH, NJ, WIN], f16, kind="ExternalInput")
    y = nc.dram_tensor("y", [B_LOC, NH, NJ, WIN], f16, kind="ExternalOutput")

    with tile.TileContext(nc) as tc:
        with (
            tc.tile_pool(name="data", bufs=2) as data_pool,
            tc.tile_pool(name="stats", bufs=2) as stats_pool,
            tc.tile_pool(name="singles", bufs=1) as singles,
        ):
            eps_tile = singles.tile([NH, 1], f32)
            nc.vector.memset(eps_tile, EPS)
            dump_g = singles.tile([NH, WIN], f16)

            state = {}

            def stats(b, js, jn, xt, ko=0):
                """per-window stats for cols [js, js+jn): bn_stats on DVE for
                the first jn-ko cols, GPSIMD accum (sum+sumsq) for the last ko
                (only in units where GPSIMD has idle ahead of the combine)."""
                tg = f"{js}_{jn}"
                kb = jn - ko
                S = stats_pool.tile([NH, kb, 2, 3], f32, tag=f"S{tg}")
                var = stats_pool.tile([NH, jn], f32, tag=f"var{tg}")
                istd = stats_pool.tile([NH, jn], f32, tag=f"istd{tg}")
                tsh = stats_pool.tile([NH, jn], f32, tag=f"tsh{tg}")
                if ko:
                    osum = stats_pool.tile([NH, ko], f32, tag=f"osum{tg}")
                    osq = stats_pool.tile([NH, ko], f32, tag=f"osq{tg}")
                    u2 = stats_pool.tile([NH, ko], f32, tag=f"u2{tg}")
                    for i in range(ko):
                        win = xt[:, js + kb + i, :]
                        nc.gpsimd.tensor_scalar(
                            out=win, in0=win, scalar1=1.0, scalar2=0.0,
                            op0=mybir.AluOpType.mult,
                            op1=mybir.AluOpType.add,
                            accum_out=osum[:, i : i + 1],
                        )
                        nc.gpsimd.scalar_tensor_tensor(
                            out=dump_g, in0=win, scalar=1.0, in1=win,
                            op0=mybir.AluOpType.mult, op1=mybir.AluOpType.mult,
                            accum_out=osq[:, i : i + 1],
                        )
                for t in range(kb):
                    nc.vector.bn_stats(out=S[:, t], in_=xt[:, js + t, :])
                # two equal 128-elem groups: mu = (m0+m1)/2,
                # 256*var = (cv0+cv1) + 64*(m0-m1)^2
                m0, m1 = S[:, :, 0, 1], S[:, :, 1, 1]
                sm = stats_pool.tile([NH, kb], f32, tag=f"sm{tg}")
                sv = stats_pool.tile([NH, kb], f32, tag=f"sv{tg}")
                d = stats_pool.tile([NH, kb], f32, tag=f"d{tg}")
                nc.vector.tensor_add(out=sm, in0=m0, in1=m1)
                nc.vector.tensor_add(out=sv, in0=S[:, :, 0, 2], in1=S[:, :, 1, 2])
                nc.vector.tensor_tensor(
                    out=d, in0=m0, in1=m1, op=mybir.AluOpType.subtract
                )
                nc.vector.tensor_mul(out=d, in0=d, in1=d)
                nc.vector.scalar_tensor_tensor(
                    out=var[:, :kb], in0=d, scalar=64.0, in1=sv,
                    op0=mybir.AluOpType.mult, op1=mybir.AluOpType.add,
                )
                if ko:
                    nc.vector.tensor_mul(out=u2, in0=osum, in1=osum)
                    nc.vector.scalar_tensor_tensor(
                        out=var[:, kb:], in0=u2, scalar=-1.0 / WIN, in1=osq,
                        op0=mybir.AluOpType.mult, op1=mybir.AluOpType.add,
                    )
                # istd = 1/sqrt(var + eps); tsh = -mu * istd
                nc.scalar.activation(
                    out=var, in_=var, func=mybir.ActivationFunctionType.Sqrt,
                    bias=eps_tile, scale=1.0 / WIN,
                )
                nc.vector.reciprocal(out=istd, in_=var)
                nc.vector.scalar_tensor_tensor(
                    out=tsh[:, :kb], in0=sm, scalar=-0.5, in1=istd[:, :kb],
                    op0=mybir.AluOpType.mult, op1=mybir.AluOpType.mult,
                )
                if ko:
                    nc.vector.scalar_tensor_tensor(
                        out=tsh[:, kb:], in0=osum, scalar=-1.0 / WIN,
                        in1=istd[:, kb:],
                        op0=mybir.AluOpType.mult, op1=mybir.AluOpType.mult,
                    )
                state[(b, js)] = (xt, istd, tsh)

            def normalize(b, js, jn, pattern, store_q):
                """normalize cols [js, js+jn) in place, then store them."""
                xt, istd, tsh = state.pop((b, js))
                for jo in range(jn):
                    win = xt[:, js + jo, :]
                    eng = pattern[jo]
                    if eng == "s":
                        nc.scalar.activation(
                            out=win,
                            in_=win,
                            func=mybir.ActivationFunctionType.Identity,
                            bias=tsh[:, jo : jo + 1],
                            scale=istd[:, jo : jo + 1],
                        )
                    else:
                        e = nc.vector if eng == "v" else nc.gpsimd
                        e.tensor_scalar(
                            out=win,
                            in0=win,
                            scalar1=istd[:, jo : jo + 1],
                            scalar2=tsh[:, jo : jo + 1],
                            op0=mybir.AluOpType.mult,
                            op1=mybir.AluOpType.add,
                        )
                store_q.dma_start(
                    out=y[b, :, js : js + jn], in_=xt[:, js : js + jn]
                )

            # software-pipelined units; small at the head (early DVE start)
            # and tail (short drain). All loads on the sync ring (in pipeline
            # order); stores on scalar, late stores on the by-then-idle sync.
            xt0 = data_pool.tile([NH, NJ, WIN], f16, tag="xt")
            xt1 = data_pool.tile([NH, NJ, WIN], f16, tag="xt")
            for xt, b, js, jn in (
                (xt0, 0, 0, 8),
                (xt0, 0, 8, 8),
                (xt0, 0, 16, 16),
                (xt0, 0, 32, 32),
                (xt0, 0, 64, 64),
                (xt1, 1, 0, 64),
                (xt1, 1, 64, 64),
            ):
                nc.sync.dma_start(
                    out=xt[:, js : js + jn], in_=x[b, :, js : js + jn]
                )
            units = [
                (0, 0, 8, 0, P8, nc.scalar),
                (0, 8, 8, 0, P8, nc.scalar),
                (0, 16, 16, 0, P16, nc.scalar),
                (0, 32, 32, 0, P32, nc.scalar),
                (0, 64, 64, 0, P64, nc.scalar),
                (1, 0, 64, 0, P64, nc.sync),
                (1, 64, 32, 0, P32, nc.scalar),
                (1, 96, 16, 0, P16L, nc.scalar),
                (1, 112, 16, 0, P16T, nc.sync),
            ]
            for b, js, jn, ko, pat, q in units:
                stats(b, js, jn, xt0 if b == 0 else xt1, ko)
                normalize(b, js, jn, pat, q)
    nc.compile()
    return nc


_NC_CACHE = None
LAST_RESULTS = None


def _get_nc():
    global _NC_CACHE
    if _NC_CACHE is None:
        _NC_CACHE = build_kernel()
    return _NC_CACHE


def kernel(x: np.ndarray) -> np.ndarray:
    global LAST_RESULTS
    assert x.shape == (B, C, H, W), x.shape
    # window-major host relayout: [B, C, H, W] -> [B, nH, nW, (c a b)] fp16
    xh = np.ascontiguousarray(
        x.astype(np.float16)
        .reshape(B, C, NH, 2, NJ, 2)
        .transpose(0, 2, 4, 1, 3, 5)
        .reshape(B, NH, NJ, WIN)
    )
    nc = _get_nc()
    in_maps = [{"x": xh[k * B_LOC : (k + 1) * B_LOC]} for k in range(N_CORES)]
    kw = {}
    if os.environ.get("KERNEL_TRACE") == "1":
        kw["trace"] = True
        if os.environ.get("KERNEL_TRACE_DIR"):
            import tempfile

            base = os.environ["KERNEL_TRACE_DIR"]
            os.makedirs(base, exist_ok=True)
            kw["tmpdir"] = tempfile.mkdtemp(dir=base)
    res = run_bass_kernel_spmd(nc, in_maps, core_ids=list(range(N_CORES)), **kw)
    LAST_RESULTS = res
    out = np.concatenate([r["y"] for r in res.results], axis=0)
    return (
        out.reshape(B, NH, NJ, C, 2, 2)
        .transpose(0, 3, 1, 4, 2, 5)
        .reshape(B, C, H, W)
        .astype(np.float32)
    )
